# revision 3
# baseline (speedup 1.0000x reference)
"""Trainium2 Bass kernel for ragged KeyQueryAttention pooling.

Math (per batch b):
    logits[t] = sum_l (x @ K)[t,l] * (x @ Q)[t,l],   t < len_b
    att = softmax(logits over valid t)
    out[b]    = sum_t att[t] * x[t, :] + bias        (sum att == 1)

Device strategy (8 NeuronCores, data-parallel over batch):
  - B=64 batches sorted by length (desc), grouped into 8 slots of 8;
    core i takes batch rank 8*j+i for slot j. One SPMD program whose
    per-slot chunk counts n_j = ceil(max_group_len/128) are compiled
    from the actual lengths (value-specialized, cached per n-tuple).
  - Host casts seq to fp16 and pre-arranges each core's slots into a
    single [128, ntot*128] image (partition = t%128, free = chunk,d),
    halving HBM traffic and giving the DMA large contiguous lines.
    Host also folds K,Q into kq = [(K+Q)/2 | (K-Q)/2] fp16 so that
    logits = ||x@U1||^2 - ||x@U2||^2 per row (diff of squares).
  - Per 8-chunk group: TensorE fp16 transposes (PSUM fp16), one DVE
    2x copy PSUM->SBUF, 8 fp16 matmuls -> g = x@[U1|U2] (PSUM fp32),
    one ScalarE Square (PSUM->SBUF fp32). Per pair of groups: one
    GpSimd subtract of the squared halves and one DVE reduce ->
    logits columns. Everything batched to amortize fixed overheads.
  - Per slot: DVE ragged mask add (-1e30), DVE row max, GpSimd
    partition_all_reduce -> replicated max, DVE negate, ScalarE exp
    (bias=-max) -> p fp16 with fp32 row sums (zrow) as accum_out,
    then n accumulating matmuls (lhsT = p column stationary, moving
    = x chunk) -> weighted-sum row [1, 128] in PSUM (no per-matmul
    128-column weight loads).
  - Outputs: outw [1, SLOTS*128] (weighted sums), outz [128, SLOTS]
    (zrow per slot). Host: out[b] = wsum/sum(zrow) + bias, un-permute.
"""

import os
import numpy as np

import concourse.bass as bass
import concourse.bacc as bacc
import concourse.tile as tile
from concourse import mybir
from concourse import bass_isa
from concourse.bass_utils import run_bass_kernel_spmd
from concourse.masks import make_identity

B, T, D, L = 64, 8192, 128, 64
NCORES = 8
SLOTS = B // NCORES
F32 = mybir.dt.float32
F16 = mybir.dt.float16
G = 8  # chunks per PSUM instruction group; subtract/reduce run per pair

LAST_EXEC_NS = None  # filled when KQA_TRACE=1

_PROG_CACHE = {}


def _build_program(n_list):
    nc = bacc.Bacc()
    ntot = sum(n_list)
    offs = [sum(n_list[:j]) for j in range(SLOTS)]

    X = nc.declare_dram_parameter("X", [128, ntot * 128], F16, isOutput=False)
    kq = nc.declare_dram_parameter("kq", [D, 2 * L], F16, isOutput=False)
    maskp = nc.declare_dram_parameter("mask", [128, ntot], F32, isOutput=False)
    outw = nc.declare_dram_parameter("outw", [1, SLOTS * 128], F32, isOutput=True)
    outz = nc.declare_dram_parameter("outz", [128, SLOTS], F32, isOutput=True)

    AF = mybir.ActivationFunctionType
    ALU = mybir.AluOpType
    AX = mybir.AxisListType

    with tile.TileContext(nc) as tc:
        with (
            tc.tile_pool(name="consts", bufs=1) as consts,
            tc.tile_pool(name="xgp", bufs=16) as xgp,
            tc.tile_pool(name="work", bufs=3) as work,
            tc.tile_pool(name="pairp", bufs=2) as pairp,
            tc.tile_pool(name="slotp", bufs=2) as slotp,
            tc.tile_pool(name="psT", bufs=2, space="PSUM") as psT,
            tc.tile_pool(name="psG", bufs=2, space="PSUM") as psG,
            tc.tile_pool(name="psW", bufs=2, space="PSUM") as psW,
        ):
            identity16 = consts.tile([128, 128], F16)
            make_identity(nc, identity16)
            kq_sb = consts.tile([D, 2 * L], F16)
            mask_sb = consts.tile([128, ntot], F32)
            logits = consts.tile([128, ntot], F32)
            outw_sb = consts.tile([1, SLOTS * 128], F32)
            outz_sb = consts.tile([128, SLOTS], F32)

            ngroups = [-(-n // G) for n in n_list]
            xg_tiles = [[] for _ in range(SLOTS)]

            def emit_dma(j, groups=None):
                n, off = n_list[j], offs[j]
                rng = range(ngroups[j]) if groups is None else groups
                for k in rng:
                    c0 = k * G
                    w = min(G, n - c0)
                    xg = xgp.tile([128, G, 128], F16, tag="xg", name=f"xg{j}_{k}")
                    nc.sync.dma_start(
                        out=xg[:, 0:w, :],
                        in_=X[:, (off + c0) * 128 : (off + c0 + w) * 128],
                    )
                    xg_tiles[j].append(xg)

            def emit_A_pair(j, pk):
                """Process PSUM groups 2*pk and 2*pk+1 of slot j."""
                n, off = n_list[j], offs[j]
                ks = [k for k in (2 * pk, 2 * pk + 1) if k < ngroups[j]]
                c0 = ks[0] * G
                sq = pairp.tile([128, 2 * G, 128], F32, tag="sq")
                wtot = 0
                for k in ks:
                    kc0 = k * G
                    w = min(G, n - kc0)
                    xg = xg_tiles[j][k]
                    xT_ps = psT.tile([128, G, 128], F16, tag="xT")
                    for i in range(w):
                        nc.tensor.transpose(xT_ps[:, i, :], xg[:, i, :], identity16)
                    xT_sb = work.tile([128, G, 128], F16, tag="xTs")
                    nc.vector.tensor_copy(xT_sb[:, 0:w, :], xT_ps[:, 0:w, :])
                    g_ps = psG.tile([128, G, 128], F32, tag="g")
                    for i in range(w):
                        nc.tensor.matmul(
                            g_ps[:, i, :], xT_sb[:, i, :], kq_sb, start=True, stop=True
                        )
                    nc.scalar.activation(
                        sq[:, wtot : wtot + w, :], g_ps[:, 0:w, :], AF.Square
                    )
                    wtot += w
                dd = pairp.tile([128, 2 * G, L], F32, tag="dd")
                nc.gpsimd.tensor_tensor(
                    dd[:, 0:wtot, :],
                    sq[:, 0:wtot, 0:L],
                    sq[:, 0:wtot, L : 2 * L],
                    op=ALU.subtract,
                )
                nc.vector.tensor_reduce(
                    logits[:, off + c0 : off + c0 + wtot],
                    dd[:, 0:wtot, :],
                    axis=AX.X,
                    op=ALU.add,
                )

            def emit_B_pre(j):
                n, off = n_list[j], offs[j]
                lm = slotp.tile([128, 64], F32, tag="lm")
                nc.vector.tensor_tensor(
                    lm[:, 0:n],
                    logits[:, off : off + n],
                    mask_sb[:, off : off + n],
                    op=ALU.add,
                )
                rowmax = slotp.tile([128, 1], F32, tag="rmax")
                nc.vector.tensor_reduce(rowmax, lm[:, 0:n], axis=AX.X, op=ALU.max)
                maxcol = slotp.tile([128, 1], F32, tag="maxc")
                nc.gpsimd.partition_all_reduce(
                    maxcol, rowmax, channels=128, reduce_op=bass_isa.ReduceOp.max
                )
                negm = slotp.tile([128, 1], F32, tag="negm")
                nc.vector.tensor_scalar_mul(negm, maxcol, -1.0)
                p_sb = slotp.tile([128, 64], F16, tag="p")
                nc.scalar.activation(
                    p_sb[:, 0:n],
                    lm[:, 0:n],
                    AF.Exp,
                    bias=negm,
                    scale=1.0,
                    accum_out=outz_sb[:, j : j + 1],
                )
                return p_sb

            def emit_B_wacc(j, p_sb):
                n = n_list[j]
                wrow = psW.tile([128, 128], F32, tag="wacc")
                for c in range(n):
                    nc.tensor.matmul(
                        wrow[0:1, :],
                        p_sb[:, c : c + 1],
                        xg_tiles[j][c // G][:, c % G, :],
                        start=(c == 0),
                        stop=(c == n - 1),
                    )
                nc.scalar.activation(
                    outw_sb[0:1, j * 128 : (j + 1) * 128], wrow[0:1, :], AF.Copy
                )

            npairs = [-(-g // 2) for g in ngroups]

            # startup: first two x groups, then weights, rest of slot 0, mask
            emit_dma(0, groups=[0, 1])
            nc.sync.dma_start(out=kq_sb, in_=kq[:, :])
            emit_dma(0, groups=range(2, ngroups[0]))
            nc.sync.dma_start(out=mask_sb, in_=maskp[:, :])
            if SLOTS > 1:
                emit_dma(1)

            p_prev = None
            for j in range(SLOTS):
                for pk in range(npairs[j]):
                    emit_A_pair(j, pk)
                    if pk == 0 and j >= 1:
                        p_prev = emit_B_pre(j - 1)
                    elif (pk == 1 or pk == npairs[j] - 1) and j >= 1 and p_prev is not None:
                        emit_B_wacc(j - 1, p_prev)
                        p_prev = None
                        if j + 1 < SLOTS:
                            emit_dma(j + 1)
                if j >= 1 and p_prev is not None:  # single-pair slot fallback
                    emit_B_wacc(j - 1, p_prev)
                    p_prev = None
                    if j + 1 < SLOTS:
                        emit_dma(j + 1)
            p_last = emit_B_pre(SLOTS - 1)
            emit_B_wacc(SLOTS - 1, p_last)
            nc.sync.dma_start(out=outw[:, :], in_=outw_sb)
            nc.sync.dma_start(out=outz[:, :], in_=outz_sb)
    nc.finalize()
    return nc


def kernel(seq, lengths, key_w, query_w, bias):
    global LAST_EXEC_NS
    seq = np.asarray(seq, dtype=np.float32)
    lengths_np = np.asarray(lengths).astype(np.int64)
    key_w = np.asarray(key_w, dtype=np.float32)
    query_w = np.asarray(query_w, dtype=np.float32)
    bias = np.asarray(bias, dtype=np.float32)

    order = np.argsort(-lengths_np, kind="stable")  # descending length
    n_list = []
    for j in range(SLOTS):
        grp = order[j * NCORES : (j + 1) * NCORES]
        n_list.append(max(1, int(-(-int(lengths_np[grp].max()) // 128))))
    key = tuple(n_list)
    if key not in _PROG_CACHE:
        _PROG_CACHE[key] = _build_program(n_list)
    nc = _PROG_CACHE[key]

    seq16 = seq.astype(np.float16)
    kqcat = np.concatenate(
        [(key_w + query_w) * 0.5, (key_w - query_w) * 0.5], axis=1
    ).astype(np.float16)

    in_maps = []
    for i in range(NCORES):
        xblocks = []
        mblocks = []
        for j, n in enumerate(n_list):
            b = int(order[j * NCORES + i])
            blk = seq16[b, : n * 128, :].reshape(n, 128, 128).transpose(1, 0, 2)
            xblocks.append(blk.reshape(128, n * 128))
            lb = int(lengths_np[b])
            col = np.where(np.arange(n * 128) < lb, 0.0, -1e30).astype(np.float32)
            mblocks.append(col.reshape(n, 128).T)
        in_maps.append(
            {
                "X": np.ascontiguousarray(np.concatenate(xblocks, axis=1)),
                "kq": kqcat,
                "mask": np.ascontiguousarray(np.concatenate(mblocks, axis=1)),
            }
        )

    trace = os.environ.get("KQA_TRACE") == "1"
    res = run_bass_kernel_spmd(
        nc, in_maps, core_ids=list(range(NCORES)), trace=trace
    )
    LAST_EXEC_NS = res.exec_time_ns

    out = np.empty((B, D), dtype=np.float32)
    for i in range(NCORES):
        rw = res.results[i]["outw"].reshape(SLOTS, 128)
        rz = res.results[i]["outz"]  # [128, SLOTS]
        for j in range(SLOTS):
            b = int(order[j * NCORES + i])
            z = rz[:, j].astype(np.float64).sum()
            out[b] = (rw[j] / z).astype(np.float32) + bias
    return out
